# revision 63
# baseline (speedup 1.0000x reference)
"""Trainium2 Bass kernel for nn_AttentionProbe_80891414053184.

Math (reference):
    y  = relu(x @ W1.T + b1)            # (B,S,H) -> (B,S,128)
    y2 = relu(y @ W2.T + b2)            # (B,S,128)
    l  = y2 @ Wq.T + pos*pos_w  (+mask) # (B,S,8) logits
    p  = softmax(l, axis=S)
    v  = y2 @ Wv.T + bv
    out[b] = sum_{s,h} p*v + bias       # (B,1)

Strategy: sequence-parallel over 8 cores (512 positions x 4 batches = 2048
tokens per core).  Each core streams its x-shard in fp8-e4m3 (W1 pre-scaled
by 64 into e4m3's normal range, the 1/64 folded into W2), runs layer 1 as
DoubleRow matmuls, the MLP tail per 512-token tile, and emits per-(batch,
head, seq-quarter) partial softmax stats (-max, Z=sum exp, W=sum exp*v) on
all 128 partitions.  The host merges the 8x16 partial stats with the
standard online-softmax combine and produces the (4,1) output.

DMA orchestration (the critical path — the kernel is HBM-stream-bound;
all of it HW-measured on the axon trn2 cores):
  * x is host-pre-swizzled to [128, B*KCH, 512] so every DMA is a
    per-partition-contiguous slab.  The whole x stream rides the gpsimd
    (SWDGE) ring as 1 MB slabs: one dominant ring sustains ~400-430 GB/s
    while three concurrently-loaded rings degrade to ~100 GB/s each, and
    1 MB slabs (vs 0.5 MB) halve the SWDGE descriptor-ring traffic that
    otherwise backlogs SDMA engines 7/15 and trickles the last ~74KB out
    over ~4us at stream end.
  * Weights/consts ride the two HWDGE rings as one DMA each (sync: w1;
    scalar: the two const blobs).  HWDGE executes FIFO per issuing
    engine with a ~2us receipt between DMAs and gets only scraps of
    bandwidth once the SWDGE stream ramps, so anything split across
    several queued HWDGE DMAs lands 10+us late and stalls the PE
    program (w1 gates every layer-1 matmul).
  * No const-gated warmup ops: the tile scheduler hoists them to the
    head of each engine's program, where a late-landing const blob
    stalls the whole engine behind a single wait.
  * Stream order is tile-major; tile t's contraction completes at
    (t+1)/4 of the stream so tiles 0-2's MLP tails hide entirely under
    the remaining stream.  Tile 3's final k-group arrives as quarter
    slabs so its matmuls start mid-group; after the last x byte only
    tile 3's tail chain (relu-W2-relu, head matmuls, softmax stats,
    stats DMA-out + completion receipt) is exposed, then the runtime's
    fixed ~8us postamble (semaphore-file restore) closes the window.

Layout notes:
  * Head projections are 32-wide column-tiled matmuls (N=128) that land
    q|v as a (128, 256) psum: lane p = 32*quarter + 8*tile + head.  The
    whole softmax-stats stage then runs on 128 partitions x 128 columns
    in three fused DVE/ACT ops.
  * Tail operands (y2, Wq/Wv blocks) are bf16: N=128 matmuls run at
    1 cyc/row in bf16 but 4 cyc/row in f32r.
"""

import numpy as np

# Problem dims (hardcoded per harness contract).
B, S, H = 4, 4096, 4096
MLP, NH = 128, 8
NCORES = 8
S_SHARD = S // NCORES        # 512 seq positions per core
TOK = B * S_SHARD            # 2048 tokens per core
NT = TOK // 512              # 4 token tiles of 512 (= one batch each)
KCH = H // 128               # 32 contraction chunks
GRP = 16                     # k-chunks per x DMA slab (1 MB fp8)
NG = KCH // GRP              # 2 slabs per tile
QT = 4                       # seq quarters per tile (512 = 4 x 128)
P32 = 32                     # lanes per psum column group (= NT * NH)

W1_SCALE = 64.0              # 2**6: lifts W1 ~N(0, 1/64^2) into e4m3 range

_cache = {}


def _build_nc(h):
    import concourse.mybir as mybir
    import concourse.tile as tile
    from concourse import bacc
    from concourse.tile import add_dep_helper

    f32 = mybir.dt.float32
    bf16 = mybir.dt.bfloat16
    fp8 = mybir.dt.float8e4
    kch = h // 128

    # Bacc (not bare Bass): its finalize() runs move_matmul_waits_to_ldweights
    # and generate_event_semaphores, which split multi-sem waits to satisfy
    # TRN2's one-wait-per-instruction encoding limit.
    nc = bacc.Bacc()
    xt_d = nc.dram_tensor("xt", [128, NT * kch, 512], fp8,
                          kind="ExternalInput")
    w1_d = nc.dram_tensor("w1s", [128, kch, MLP], fp8, kind="ExternalInput")
    # cwb: bf16 blob = [w2t (128) | per-tile zero-padded 32-wide head blocks
    # [wq32 x4 | wv32 x4] (256)]; cf: f32 blob = [l-add (pos*pos_w + mask -
    # shift) 128 | bv 1 | b1 1 | b2 1] per lane/row.
    cwb_d = nc.dram_tensor("cwb", [MLP, MLP + 2 * P32 * NT], bf16,
                           kind="ExternalInput")
    cf_d = nc.dram_tensor("cf", [128, 128 + 3], f32, kind="ExternalInput")
    st_d = nc.dram_tensor("stats", [128, 3], f32, kind="ExternalOutput")

    AF = mybir.ActivationFunctionType
    OP = mybir.AluOpType
    PM = mybir.MatmulPerfMode.DoubleRow

    # Stream order: tile-major for tiles 0-2 with tile 3's first half
    # pulled into mid-stream — its 1.7us of layer-1 then fills the PE's
    # mid-stream arrival stalls instead of landing after the last byte.
    # Tiles 0..2's tails (each ~3us of ACT/PE/DVE ping-pong latency)
    # hide under the remaining stream; after the last byte only tile 3's
    # final quarter-slab matmuls and its tail chain are exposed.
    order = [(0, 0), (0, 1), (1, 0), (3, 0), (1, 1), (2, 0), (2, 1)]
    assert NT == 4 and NG == 2
    # Queue per slab: measured on HW, one dominant ring streams at
    # ~390-430 GB/s while three concurrently-active rings degrade to
    # ~100 GB/s each (~300 total) — the per-packet queue round-robin
    # wastes ~25%.  So the whole x stream stays on the gpsimd (SWDGE)
    # ring; the sync (HWDGE) ring carries w1+consts up front (they gate
    # every layer-1 matmul, and a starved side-queue load measurably
    # serializes the whole PE program) and the stats output at the end.
    # 1 MB slabs halve the SWDGE descriptor-ring traffic vs 0.5 MB —
    # that ring shares AXI ports with SDMA engines 7/15, whose backlog
    # is what trickled the last ~74KB out over ~4us at stream end.
    qnames = ["gpsimd"] * len(order)

    with tile.TileContext(nc) as tc:
        with (
            tc.tile_pool(name="const", bufs=1) as const,
            tc.tile_pool(name="xp", bufs=1) as xp,
            tc.tile_pool(name="yp", bufs=2) as yp,
            tc.tile_pool(name="y2p", bufs=2) as y2p,
            tc.tile_pool(name="smallp", bufs=1) as smallp,
            tc.tile_pool(name="statsp", bufs=1) as statsp,
            tc.tile_pool(name="ps_y", bufs=4, space="PSUM") as ps_y,
            tc.tile_pool(name="ps_y2", bufs=1, space="PSUM") as ps_y2,
            tc.tile_pool(name="ps_qv", bufs=1, space="PSUM") as ps_qv,
        ):
            eng = {"sync": nc.sync, "scalar": nc.scalar, "gpsimd": nc.gpsimd}
            x_sb = {}

            def x_dma(t, g, e, nsub=1):
                # nsub>1 splits the slab into sub-DMAs so its matmuls can
                # start as the first sub lands (used for the first slab —
                # an earlier PE start — and the last — a shorter exposed
                # tail).  e may be a list of engines, one per sub.
                sub = GRP // nsub
                engines = e if isinstance(e, list) else [e] * nsub
                subs = []
                for j in range(nsub):
                    sl = xp.tile([128, sub, 512], fp8, tag=f"x{t}_{g}_{j}",
                                 name=f"x{t}_{g}_{j}")
                    c0 = t * kch + g * GRP + j * sub
                    engines[j].dma_start(out=sl[:], in_=xt_d[:, c0:c0 + sub, :])
                    subs.append(sl)
                x_sb[(t, g)] = subs

            # DMA-queue plan (HWDGE queues get only scraps of bandwidth
            # once the SWDGE stream ramps, and execute FIFO per engine
            # with a ~2us receipt between DMAs — so each carries only
            # what's needed in its fast first-second):
            #   sync:   first two quarter-slabs of x (PE's earliest work,
            #           landing ~1.5us before the SWDGE stream warms up),
            #           stats out at the end.
            #   scalar: w1 (gates all layer-1), then cwb/cf (gate tails).
            #   gpsimd: everything else — the bulk x stream.
            # All weights/consts on the scalar (HWDGE) ring so the sync
            # ring carries nothing early: a loaded sync queue pre-empts
            # the SWDGE x stream (strict priority) and delays its start
            # by ~1.5us.  The scalar ring's ACT-table load overlaps its
            # DMA issue, and HWDGE priority still lands w1 by ~10.5us.
            w1_sb = const.tile([128, kch, MLP], fp8)
            nc.scalar.dma_start(out=w1_sb[:], in_=w1_d[:])
            # first slab unsplit: a split's extra gpsimd DMA deepens the
            # semaphore-lane reuse chain, delaying the final sub-slabs'
            # emissions (and the last data+sem) by ~1.5us — more than the
            # earlier start it buys.  The spin chain alone removes the
            # dispatch lag at this slab's semaphore.
            x_dma(*order[0], eng[qnames[0]])
            cwb_sb = const.tile([MLP, MLP + 2 * P32 * NT], bf16)
            nc.scalar.dma_start(out=cwb_sb[:], in_=cwb_d[:])
            cf_sb = const.tile([128, 128 + 3], f32)
            nc.scalar.dma_start(out=cf_sb[:], in_=cf_d[:])
            x_dma(*order[1], eng[qnames[1]])
            cwr_sb = cwb_sb[:, 0:MLP]
            cwh_sb = cwb_sb[:, MLP:MLP + 2 * P32 * NT]
            ca2_sb = cf_sb[:, 0:129]
            cb_sb = cf_sb[:, 129:131]
            # remaining full slabs.
            for i in range(2, len(order)):
                x_dma(*order[i], eng[qnames[i]])

            stats_sb = statsp.tile([128, 3], f32)
            # col 0 is unused on-chip (the host merge supplies the shifts)
            nc.vector.memset(stats_sb[:, 0:1], 0.0)
            # Dummy write to the output tensor, mid-stream: warms the HBM
            # write path (row/page state) so the final stats DMA's
            # completion receipt — serial on the critical path — is the
            # ~0.2us warm case instead of the ~2us cold case.
            nc.sync.dma_start(out=st_d[:, 0:1], in_=stats_sb[:, 0:1])
            # zw memset first on DVE — it gates the qv group opener and
            # must not queue behind any const-DMA-gated warmup.
            zw_sb = const.tile([128, 512], bf16)
            nc.vector.memset(zw_sb[:], 0.0)

            psum_y = []
            for t in range(NT):
                y_ps = ps_y.tile([128, 512], f32, tag="y", name=f"y_ps{t}")
                psum_y.append(y_ps)

            qv_ps = ps_qv.tile([128, 512], f32, tag="qv", name="qv_ps")
            # skip_group_check: the sim's zero-region group checker cannot
            # express column-strip accumulation within one bank; the actual
            # per-element has_written semantics (opener sets all bits, strips
            # accumulate) are still simulated and are what hardware does.
            zero_mm = nc.tensor.matmul(qv_ps[:, 0:512], zw_sb[:, 0:128],
                                       zw_sb[:, 0:512], start=True, stop=False,
                                       skip_group_check=True)

            # --- PE spin chain: every engine pays a ~2us event-wake
            # penalty on a data-gated wait it actually sleeps on (HW-
            # measured: warm_pe fires ~2.3us after its w1 sem; the first
            # layer-1 matmul ~2us after its slab sem).  These tiny
            # dependency-free matmuls keep the PE dispatching from the
            # zero-opener (~8.7us) until the first x sub-slab lands
            # (~11.5-12.5us), so the w1 warmup and the first layer-1
            # matmul fire with a hot dispatcher instead of waking.
            spin_ps = ps_y2.tile([128, 512], f32, tag="y2", name="spin_ps")
            for i in range(18):
                nc.tensor.matmul(spin_ps[0:8, 8 * i:8 * i + 8],
                                 zw_sb[:, 0:8], zw_sb[:, 8 * i:8 * i + 8],
                                 start=True, stop=True)

            # --- Warmup / staging: each engine observes every const-DMA lane
            # once, so steady-state instructions carry at most one new wait
            # (fewer split-events from Bacc's generate_event_semaphores).
            # Only the w1 warmup precedes the k-loop; everything gated on
            # the cwb/cf blobs is emitted after tile 0's layer-1 so a
            # slow-trickling const DMA can never stall the stream.
            warm_ps = ps_y2.tile([128, 512], f32, tag="y2", name="warm_ps")
            warm_pe_last = nc.tensor.matmul(warm_ps[:, 0:NH], w1_sb[:, 0, :],
                                            w1_sb[:, 0, 0:NH],
                                            start=True, stop=True)

            # Layer 1 is DoubleRow fp8: one matmul consumes 2 k-chunks.
            def l1_mms(t, g):
                subs = x_sb[(t, g)]
                sub = GRP // len(subs)
                for kk in range(0, GRP, 2):
                    k = g * GRP + kk
                    sl = subs[kk // sub]
                    mm = nc.tensor.matmul(
                        psum_y[t][:],
                        w1_sb[:, k:k + 2, :],
                        sl[:, kk % sub:kk % sub + 2, :],
                        start=(k == 0), stop=(k + 2 == kch),
                        perf_mode=PM,
                    )
                    if k == 0 and t == 0:
                        add_dep_helper(mm.ins, warm_pe_last.ins, sync=False,
                                       reason="warmups before first mm")

            def tail(t):
                # relu1 runs as two 256-col halves on alternating engines
                # (h0: ACT, h1: DVE) so they go in parallel; W2 runs as
                # two 256-col halves so relu1-h0 -> W2-h0 -> relu2-h0
                # pipelines against relu1-h1 (tile 2's and 3's tails are
                # both partially exposed at stream end, so the halved
                # chain latency matters; a merged 512-col W2 measured
                # ~5us worse end-to-end).  Each half's W2 psum is its own
                # bank (psum slots are bank-granular) so the two
                # start/stop groups don't collide.
                y_sb = yp.tile([128, 512], bf16, tag="ysb", name=f"y_sb{t}")
                y2_sb = y2p.tile([128, 512], bf16, tag="y2sb",
                                 name=f"y2_sb{t}")
                if t == NT - 1:
                    # exposed tile: 128-col quarters, relu engines
                    # alternating, W2 psums recycled from the (now dead)
                    # layer-1 banks of tiles 0..2 via the ps_y pool —
                    # shortens the exposed chain and lets each quarter's
                    # head matmuls fire right after its own relu2.
                    for qq in range(4):
                        cs = slice(128 * qq, 128 * (qq + 1))
                        if qq % 2 == 0:
                            nc.scalar.activation(out=y_sb[:, cs],
                                                 in_=psum_y[t][:, cs],
                                                 func=AF.Relu,
                                                 bias=cb_sb[:, 0:1],
                                                 scale=1.0)
                        else:
                            nc.vector.tensor_scalar(out=y_sb[:, cs],
                                                    in0=psum_y[t][:, cs],
                                                    scalar1=cb_sb[:, 0:1],
                                                    scalar2=0.0, op0=OP.add,
                                                    op1=OP.max)
                        y2q_ps = ps_y.tile([128, 128], f32, tag="y",
                                           name=f"y2q{qq}")
                        nc.tensor.matmul(y2q_ps[:], cwr_sb[:], y_sb[:, cs],
                                         start=True, stop=True)
                        if qq % 2 == 0:
                            nc.vector.tensor_scalar(out=y2_sb[:, cs],
                                                    in0=y2q_ps[:],
                                                    scalar1=cb_sb[:, 1:2],
                                                    scalar2=0.0, op0=OP.add,
                                                    op1=OP.max)
                        else:
                            nc.scalar.activation(out=y2_sb[:, cs],
                                                 in_=y2q_ps[:],
                                                 func=AF.Relu,
                                                 bias=cb_sb[:, 1:2],
                                                 scale=1.0)
                    hrange = []
                else:
                    hrange = [0, 1]
                for hh in hrange:
                    cs = slice(256 * hh, 256 * (hh + 1))
                    if hh == 0:
                        nc.scalar.activation(out=y_sb[:, cs],
                                             in_=psum_y[t][:, cs],
                                             func=AF.Relu,
                                             bias=cb_sb[:, 0:1], scale=1.0)
                    else:
                        nc.vector.tensor_scalar(out=y_sb[:, cs],
                                                in0=psum_y[t][:, cs],
                                                scalar1=cb_sb[:, 0:1],
                                                scalar2=0.0, op0=OP.add,
                                                op1=OP.max)
                    y2_ps = ps_y2.tile([128, 256], f32, tag=f"y2h{hh}",
                                       name=f"y2_ps{t}_{hh}", bufs=1)
                    nc.tensor.matmul(y2_ps[:], cwr_sb[:], y_sb[:, cs],
                                     start=True, stop=True)
                    if hh == 0:
                        nc.vector.tensor_scalar(out=y2_sb[:, cs],
                                                in0=y2_ps[:],
                                                scalar1=cb_sb[:, 1:2],
                                                scalar2=0.0, op0=OP.add,
                                                op1=OP.max)
                    else:
                        nc.scalar.activation(out=y2_sb[:, cs], in_=y2_ps[:],
                                             func=AF.Relu,
                                             bias=cb_sb[:, 1:2], scale=1.0)
                # Head projections: block t is zero outside rows 8t..8t+8, so
                # tiles t=0..3 accumulate into lane group 8t+h of each
                # 32-partition column strip qt; lane p = 32*qt + 8*t + h.
                for qt in range(QT):
                    mm = nc.tensor.matmul(
                        qv_ps[32 * qt:32 * (qt + 1), 0:128],
                        cwh_sb[:, P32 * t:P32 * (t + 1)],
                        y2_sb[:, 128 * qt:128 * (qt + 1)],
                        start=False, stop=False,
                        tile_position=(0, 32 * qt),
                        skip_group_check=True)
                    if t == 0:
                        add_dep_helper(mm.ins, zero_mm.ins, sync=False,
                                       reason="group opener before accum")
                for qt in range(QT):
                    mm = nc.tensor.matmul(
                        qv_ps[32 * qt:32 * (qt + 1), 128:256],
                        cwh_sb[:, P32 * NT + P32 * t:P32 * NT + P32 * (t + 1)],
                        y2_sb[:, 128 * qt:128 * (qt + 1)],
                        start=False, stop=(t == NT - 1),
                        tile_position=(0, 32 * qt),
                        skip_group_check=True)
                    if t == 0:
                        add_dep_helper(mm.ins, zero_mm.ins, sync=False,
                                       reason="group opener before accum")

            # Consume slabs in arrival order; fire each tile's tail right
            # after its last k-group — EXCEPT tile 2's, deferred past
            # tile 3's final quarter-slab matmuls: its PE ops (W2/head
            # matmuls, which ping-pong with ACT/DVE and idle the PE
            # between hops) would otherwise queue ahead of them in PE
            # program order.  Its ACT/DVE relus still run as soon as its
            # psum stops.
            done = {t: 0 for t in range(NT)}
            for (t, g) in order:
                l1_mms(t, g)
                done[t] += 1
                if done[t] == NG and t != NT - 2:
                    tail(t)
            # tile 3's final k-group arrives as quarter-slab DMAs so the
            # last tile's matmuls and tail start mid-group.
            t = NT - 1
            g = NG - 1
            x_dma(t, g, nc.gpsimd, nsub=4)
            l1_mms(t, g)
            tail(NT - 2)
            tail(t)

            # --- softmax stats on (128, 128).  The host pre-subtracts each
            # lane's known position-bias max from cf, so l' = q + add - s is
            # bounded (|l'| < ~15 << 88) and exp needs no on-chip max
            # subtraction; the host merge uses s as the online-softmax max.
            # A: l' = q + (add - s)
            l_sb = smallp.tile([128, 128], f32, tag="l", name="l_sb")
            nc.vector.tensor_add(out=l_sb[:], in0=qv_ps[:, 0:128],
                                 in1=ca2_sb[:, 0:128])
            # C: e = exp(l');  stats1 = Z = sum e
            e_sb = smallp.tile([128, 128], f32, tag="e", name="e_sb")
            nc.scalar.activation(out=e_sb[:], in_=l_sb[:], func=AF.Exp,
                                 bias=0.0, scale=1.0,
                                 accum_out=stats_sb[:, 1:2])
            # D: ev = (v + bv) * e;  stats2 = W = sum ev
            ev_sb = smallp.tile([128, 128], f32, tag="ev", name="ev_sb")
            nc.vector.scalar_tensor_tensor(
                out=ev_sb[:], in0=qv_ps[:, 128:256], scalar=ca2_sb[:, 128:129],
                in1=e_sb[:], op0=OP.add, op1=OP.mult,
                accum_out=stats_sb[:, 2:3])

            nc.sync.dma_start(out=st_d[:], in_=stats_sb[:])

    nc.finalize()
    return nc


def get_nc(h=H):
    if h not in _cache:
        _cache[h] = _build_nc(h)
    return _cache[h]


def make_core_inputs(x, mask, W1, b1, W2, b2, Wq, Wv, bv, pos_w, bias):
    """Host-side shard + transpose. Returns list of 8 in_maps."""
    import ml_dtypes
    h = x.shape[2]
    kch = h // 128
    # W1 scaled up by 64 into e4m3's normal range; layer-1 output then
    # carries a 64x factor, removed by folding 1/64 into W2 (and 64 into b1,
    # since relu(64a) = 64 relu(a) commutes with the positive scale).
    w1s = np.ascontiguousarray(
        (W1 * W1_SCALE).reshape(MLP, kch, 128).transpose(2, 1, 0)).astype(
            ml_dtypes.float8_e4m3)
    cwb = np.zeros((MLP, MLP + 2 * P32 * NT), dtype=np.float32)
    cwb[:, 0:MLP] = W2.T / W1_SCALE
    # zero-padded per-tile head blocks: block t covers psum rows 8t..8t+8
    for t in range(NT):
        cwb[:, MLP + P32 * t + NH * t:MLP + P32 * t + NH * (t + 1)] = Wq.T
        cwb[:, MLP + P32 * NT + P32 * t + NH * t:
             MLP + P32 * NT + P32 * t + NH * (t + 1)] = Wv.T
    cwb = cwb.astype(ml_dtypes.bfloat16)
    pos = np.arange(S, dtype=np.float32)
    maskadd = np.where(mask == 0, np.float32(-1e9), np.float32(0.0))  # (B,S)

    in_maps = []
    shifts = []
    for c in range(NCORES):
        sl = slice(c * S_SHARD, (c + 1) * S_SHARD)
        # x-slab layout [p, t*kch + kc, n]: value = x[t, qt*128+n ... ] with
        # contraction row kc*128+p, token index (within shard) split later
        # into quarters by the head matmuls; layer 1 consumes it flat.
        xs = x[:, sl, :].astype(ml_dtypes.float8_e4m3)      # (B, 512, H)
        xt = np.ascontiguousarray(
            xs.reshape(NT, 512, kch, 128).transpose(3, 0, 2, 1))
        # (128, NT, kch, 512) -> [128, NT*kch, 512]
        xt = xt.reshape(128, NT * kch, 512)
        # stats lane p = 32*qt + 8*t + h covers tokens qt*128.. of batch t
        cf = np.empty((128, 128 + 3), dtype=np.float32)
        # lane (qt, t, h), col n -> pos qt*128+n, batch t
        posq = pos[sl].reshape(QT, 128)                      # (QT, 128)
        madd = maskadd[:, sl].reshape(NT, QT, 128)           # (T, QT, 128)
        lane_add = (pos_w.astype(np.float32)[None, None, :, None] *
                    posq[:, None, None, :] +
                    madd.transpose(1, 0, 2)[:, :, None, :])  # (QT, T, NH, 128)
        la = lane_add.reshape(128, 128)
        s_lane = la.max(axis=1)              # per-lane shift (host-known)
        cf[:, 0:128] = la - s_lane[:, None]
        cf[:, 128] = np.tile(bv.astype(np.float32), QT * NT)
        cf[:, 129] = b1.astype(np.float32) * W1_SCALE
        cf[:, 130] = b2.astype(np.float32)
        shifts.append(s_lane.astype(np.float64))
        in_maps.append({"xt": xt, "w1s": w1s, "cwb": cwb, "cf": cf})
    return in_maps, np.stack(shifts)


def merge_stats(stats_all, shifts, bias):
    """stats_all: (NCORES, 128, 3), lane 32*qt+8*t+h with [unused, Z, W]
    relative to the host-known per-lane shift -> (B, 1) output."""
    st = np.asarray(stats_all, dtype=np.float64).reshape(NCORES * QT, NT, NH, 3)
    m = np.asarray(shifts, dtype=np.float64).reshape(NCORES * QT, NT, NH)
    Z = st[..., 1]
    W = st[..., 2]
    M = m.max(axis=0)        # (B, NH)
    alpha = np.exp(m - M[None])
    Zg = (alpha * Z).sum(axis=0)
    Wg = (alpha * W).sum(axis=0)
    out = (Wg / Zg).sum(axis=1)          # (B,)
    return (out[:, None] + np.float64(bias.reshape(1)[0])).astype(np.float32)


def kernel(x, mask, W1, b1, W2, b2, Wq, Wv, bv, pos_w, bias, _trace=False):
    from concourse.bass_utils import run_bass_kernel_spmd

    x = np.asarray(x, dtype=np.float32)
    in_maps, shifts = make_core_inputs(
        x, np.asarray(mask), *(np.asarray(a) for a in
                               (W1, b1, W2, b2, Wq, Wv, bv, pos_w, bias)))
    nc = get_nc()
    res = run_bass_kernel_spmd(nc, in_maps, core_ids=list(range(NCORES)),
                               trace=_trace)
    stats_all = np.stack([r["stats"] for r in res.results])  # (C, 128, 3)
    out = merge_stats(stats_all, shifts, np.asarray(bias))
    if _trace:
        kernel.last_result = res
    return out
